# revision 1
# baseline (speedup 1.0000x reference)
"""Multi-head cross-attention Trainium2 kernel (8 NeuronCores, SPMD).

Problem: nn_MultiHeadCrossAttention_31791347925263
  x:[4,2048,768], y:[4,2048,768], 12 heads x 64, fp32.
  out = softmax((x Wq^T)(y Wk^T)^T / 8 + mask) (y Wv^T) Wo^T   (+ zero biases)

Sharding: 8 cores = (batch b in 0..3) x (query half in 0..1). Each core
computes the full attention for its 1024 query rows against all 2048 keys
of its batch. No collectives; outputs concatenate.

Per-core dataflow (all matmuls in float32r = TF32-like, 11-bit mantissa RNE):
  host:  xT=[768,1024], yT=[768,2048], WqT/WkT/WvT/WoT = W.T contiguous
         (k/v rows of Wkv are interleaved per head: 64 k then 64 v per 128)
  kT  = WkT-blocks^T-matmul yT      -> [768(k-dim), 2048(sk)]
  v'  = yT-blocks^T-matmul WvT      -> [2048(sk), 780] (65 cols/head: v|ones)
  qT  = WqT-blocks^T-matmul xT      -> [768(q-dim), 1024(sq)]
  per head pair (2*hb, 2*hb+1), per sk-block (128 keys):
      S^T = kT_h-block^T-matmul qT_h      -> PSUM [128, 1024] (row groups
                                             0-63/64-127 alternate -> the two
                                             heads' K=64 matmuls overlap)
      P~  = exp(S^T * 0.125)              -> SBUF f32r (ACT, no max-subtract:
                                             scores ~ N(0,1), max ~ 4)
      valT_h += v'[skb,h]^T-matmul P~     -> PSUM [65, 1024]
                                             (row 64 = softmax denominator)
  valnorm_h = valT_h[0:64] * bcast(1/valT_h[64])
      (DVE copy + fast-reciprocal, GPSIMD partition-broadcast; valnorm
       overwrites qT's tiles - same [128,1024] f32r shape, qT[hb] is dead
       once pair hb's QK matmuls are done)
  o[sqb]    = valnorm-blocks^T-matmul WoT -> [1024, 768] -> DMA out

All matmul outputs except valT share one 2-buf [128,1024] PSUM tag
(2 banks per slot; valT pool 2x2 banks) = exactly the 8 PSUM banks.
"""

import numpy as np

B, S, D = 4, 2048, 768
H, Dh = 12, 64
SQ = S // 2          # queries per core
N_CORES = 8
DB = D // 128        # 6 d_model blocks
SKB = S // 128       # 16 key blocks
SQB = SQ // 128      # 8 query blocks per core
VPW = H * (Dh + 1)   # 780: v' width (64 v cols + 1 ones col per head)

_cache = {}


def _build_nc():
    import concourse.mybir as mybir
    import concourse.tile as tile
    from concourse import bacc

    f32 = mybir.dt.float32
    f32r = mybir.dt.float32r
    EXP = mybir.ActivationFunctionType.Exp

    nc = bacc.Bacc("TRN2", target_bir_lowering=False)
    xT = nc.dram_tensor("xT", [D, SQ], f32, kind="ExternalInput")
    yT = nc.dram_tensor("yT", [D, S], f32, kind="ExternalInput")
    WqT = nc.dram_tensor("WqT", [D, D], f32, kind="ExternalInput")
    WkT = nc.dram_tensor("WkT", [D, D], f32, kind="ExternalInput")
    WvT = nc.dram_tensor("WvT", [D, D], f32, kind="ExternalInput")
    WoT = nc.dram_tensor("WoT", [D, D], f32, kind="ExternalInput")
    out = nc.dram_tensor("out", [SQ, D], f32, kind="ExternalOutput")

    with tile.TileContext(nc) as tc:
        with tc.tile_pool(name="persist", bufs=1) as pp, \
             tc.tile_pool(name="mmps", bufs=2, space="PSUM") as mm_ps, \
             tc.tile_pool(name="vtp", bufs=2, space="PSUM") as vt_ps:

            def mm_tile(cols):
                return mm_ps.tile([128, cols], f32, name="mmps", tag="mmps",
                                  padded_shape=[128, SQ])

            kT = [pp.tile([128, S], f32r, name=f"kT{i}") for i in range(DB)]
            vp = [pp.tile([128, VPW], f32r, name=f"vp{i}") for i in range(SKB)]
            qT = [pp.tile([128, SQ], f32r, name=f"qT{i}") for i in range(DB)]
            vnorm = qT  # valnorm overwrites qT (same shape; see docstring)

            with tc.tile_pool(name="ld_y", bufs=1) as ld_y:
                yTs = [ld_y.tile([128, S], f32r, name=f"yTs{i}")
                       for i in range(DB)]

                # ---- kT projection: kT[ob] = (WkT col-block)^T @ yT ----
                with tc.tile_pool(name="ld_wk", bufs=1) as ld_wk:
                    wkTs = [ld_wk.tile([128, D], f32r, name=f"wkTs{i}")
                            for i in range(DB)]
                    for i in range(DB):
                        nc.sync.dma_start(
                            out=wkTs[i],
                            in_=WkT[i * 128:(i + 1) * 128, :].bitcast(f32r))
                    for c4 in range(4):
                        for i in range(DB):
                            nc.sync.dma_start(
                                out=yTs[i][:, c4 * 512:(c4 + 1) * 512],
                                in_=yT[i * 128:(i + 1) * 128,
                                       c4 * 512:(c4 + 1) * 512].bitcast(f32r))
                    wvTs = [ld_y.tile([128, D], f32r, name=f"wvTs{i}")
                            for i in range(DB)]
                    for i in range(DB):
                        nc.sync.dma_start(
                            out=wvTs[i],
                            in_=WvT[i * 128:(i + 1) * 128, :].bitcast(f32r))
                    # nc4 outer: the first 6 groups need only yT column
                    # chunk 0, so compute starts while chunks 1-3 stream in
                    for nc4 in range(4):
                        for ob in range(DB):
                            ps = mm_tile(512)
                            for kb in range(DB):
                                nc.tensor.matmul(
                                    ps[:, :],
                                    wkTs[kb][:, ob * 128:(ob + 1) * 128],
                                    yTs[kb][:, nc4 * 512:(nc4 + 1) * 512],
                                    start=(kb == 0), stop=(kb == DB - 1))
                            nc.vector.tensor_copy(
                                kT[ob][:, nc4 * 512:(nc4 + 1) * 512], ps[:, :])

                # ---- v' projection: v[skb] = (yT blk)^T @ WvT ----
                if True:
                    for skb in range(SKB):
                        vps3 = vp[skb].rearrange("p (h c) -> p h c", c=Dh + 1)
                        nc.vector.memset(vps3[:, :, Dh].bitcast(f32), 1.0)
                        for nc2 in range(2):
                            n0, n1 = nc2 * 512, min(D, (nc2 + 1) * 512)
                            ps = mm_tile(512)
                            for kb in range(DB):
                                nc.tensor.matmul(
                                    ps[:, 0:n1 - n0],
                                    yTs[kb][:, skb * 128:(skb + 1) * 128],
                                    wvTs[kb][:, n0:n1],
                                    start=(kb == 0), stop=(kb == DB - 1))
                            # contiguous v-cols -> 65-strided layout
                            src = ps[:, 0:n1 - n0].rearrange(
                                "p (h c) -> p h c", c=Dh)
                            dst = vps3[:, nc2 * 8:nc2 * 8 + (n1 - n0) // Dh,
                                       0:Dh]
                            nc.vector.tensor_copy(dst, src)

            # ---- qT projection ----
            with tc.tile_pool(name="ld_x", bufs=1) as ld_x:
                xTs = [ld_x.tile([128, SQ], f32r, name=f"xTs{i}")
                       for i in range(DB)]
                wqTs = [ld_x.tile([128, D], f32r, name=f"wqTs{i}")
                        for i in range(DB)]
                # critical-path order: weights, then xT halves in chunk order
                for i in range(DB):
                    nc.sync.dma_start(
                        out=wqTs[i],
                        in_=WqT[i * 128:(i + 1) * 128, :].bitcast(f32r))
                for c2 in range(2):
                    for i in range(DB):
                        nc.sync.dma_start(
                            out=xTs[i][:, c2 * 512:(c2 + 1) * 512],
                            in_=xT[i * 128:(i + 1) * 128,
                                   c2 * 512:(c2 + 1) * 512].bitcast(f32r))
                for nc2 in range(2):
                    for ob in range(DB):
                        ps = mm_tile(512)
                        for kb in range(DB):
                            nc.tensor.matmul(
                                ps[:, :],
                                wqTs[kb][:, ob * 128:(ob + 1) * 128],
                                xTs[kb][:, nc2 * 512:(nc2 + 1) * 512],
                                start=(kb == 0), stop=(kb == DB - 1))
                        nc.vector.tensor_copy(
                            qT[ob][:, nc2 * 512:(nc2 + 1) * 512], ps[:, :])

            # ---- attention ----
            with tc.tile_pool(name="late", bufs=1) as lp:
                woT = [lp.tile([128, D], f32r, name=f"woT{i}")
                       for i in range(DB)]
                for i in range(DB):
                    nc.sync.dma_start(
                        out=woT[i],
                        in_=WoT[i * 128:(i + 1) * 128, :].bitcast(f32r))

                with tc.tile_pool(name="psb", bufs=5) as p_pool, \
                     tc.tile_pool(name="nrm", bufs=2) as nrm_pool:
                    for hb in range(H // 2):
                        h0, h1 = 2 * hb, 2 * hb + 1
                        vt0 = vt_ps.tile([65, SQ], f32, name="valT")
                        vt1 = vt_ps.tile([65, SQ], f32, name="valT")
                        for skb in range(SKB):
                            st0 = mm_tile(SQ)
                            st1 = mm_tile(SQ)
                            for j in range(2):
                                for r0, st in ((0, st0), (64, st1)):
                                    nc.tensor.matmul(
                                        st[:, j * 512:(j + 1) * 512],
                                        kT[hb][r0:r0 + 64,
                                               skb * 128:(skb + 1) * 128],
                                        qT[hb][r0:r0 + 64,
                                               j * 512:(j + 1) * 512],
                                        start=True, stop=True)
                            pt0 = p_pool.tile([128, SQ], f32r, name="pT")
                            pt1 = p_pool.tile([128, SQ], f32r, name="pT")
                            nc.scalar.activation(pt0[:, :], st0[:, :], EXP,
                                                 scale=0.125)
                            nc.scalar.activation(pt1[:, :], st1[:, :], EXP,
                                                 scale=0.125)
                            for h, vt, pt in ((h0, vt0, pt0), (h1, vt1, pt1)):
                                for j in range(2):
                                    nc.tensor.matmul(
                                        vt[:, j * 512:(j + 1) * 512],
                                        vp[skb][:, h * 65:h * 65 + 65],
                                        pt[:, j * 512:(j + 1) * 512],
                                        start=(skb == 0),
                                        stop=(skb == SKB - 1))
                        for h, vt in ((h0, vt0), (h1, vt1)):
                            r0 = (h % 2) * 64
                            # single fast copy frees the PSUM accumulator so
                            # the next pair's PV can start immediately
                            vals = nrm_pool.tile([65, SQ], f32, name="vals")
                            nc.vector.tensor_copy(vals[:, :], vt[:, :])
                            rec = nrm_pool.tile([1, SQ], f32, name="rec")
                            nc.vector.reciprocal(rec[:, :], vals[64:65, :])
                            rbc = nrm_pool.tile([64, SQ], f32, name="rbc")
                            nc.gpsimd.partition_broadcast(rbc[:, :], rec[:, :])
                            nc.vector.tensor_mul(
                                vnorm[hb][r0:r0 + 64, :], vals[0:64, :],
                                rbc[:, :])

                # ---- output projection ----
                # alternate PSUM slots between the mm pool and the (now idle)
                # valT pool -> 4 concurrent accumulation groups instead of 2
                with tc.tile_pool(name="osb", bufs=3) as o_pool:
                    for sqb in range(SQB):
                        if sqb % 2 == 0:
                            op = mm_tile(D)
                        else:
                            op = vt_ps.tile([128, D], f32, name="valT",
                                            tag="valT",
                                            padded_shape=[128, SQ])
                        for nc2 in range(2):
                            n0, n1 = nc2 * 512, min(D, (nc2 + 1) * 512)
                            for kb in range(DB):
                                nc.tensor.matmul(
                                    op[:, n0:n1],
                                    vnorm[kb][:, sqb * 128:(sqb + 1) * 128],
                                    woT[kb][:, n0:n1],
                                    start=(kb == 0), stop=(kb == DB - 1))
                        ot = o_pool.tile([128, D], f32, name="osb")
                        nc.vector.tensor_copy(ot[:, :], op[:, :])
                        nc.sync.dma_start(
                            out=out[sqb * 128:(sqb + 1) * 128, :], in_=ot[:, :])

    nc.compile()
    return nc


def _get_nc():
    if "nc" not in _cache:
        _cache["nc"] = _build_nc()
    return _cache["nc"]


def _host_fallback(x, y, mask, Wq, bq, Wkv, bkv, Wo, bo):
    Bb, Ss, _ = x.shape
    q = x @ Wq.T + bq
    kv = y @ Wkv.T + bkv
    q = q.reshape(Bb, Ss, H, Dh).transpose(0, 2, 1, 3)
    kv = kv.reshape(Bb, Ss, H, 2 * Dh).transpose(0, 2, 1, 3)
    k, v = kv[..., :Dh], kv[..., Dh:]
    scaled = np.einsum("bhqd,bhkd->bhqk", q, k) / np.sqrt(np.float32(Dh))
    scaled = scaled + mask
    scaled -= scaled.max(axis=-1, keepdims=True)
    e = np.exp(scaled)
    attn = e / e.sum(axis=-1, keepdims=True)
    values = np.einsum("bhqk,bhkd->bhqd", attn, v)
    values = values.transpose(0, 2, 1, 3).reshape(Bb, Ss, H * Dh)
    return (values @ Wo.T + bo).astype(np.float32)


def _run(inputs, trace=False, trace_cores=None):
    """Returns (full_output, BassKernelResults)."""
    from concourse.bass_utils import run_bass_kernel_spmd

    x = np.ascontiguousarray(np.asarray(inputs["x"], dtype=np.float32))
    y = np.ascontiguousarray(np.asarray(inputs["y"], dtype=np.float32))
    Wq = np.asarray(inputs["Wq"], dtype=np.float32)
    Wkv = np.asarray(inputs["Wkv"], dtype=np.float32)
    Wo = np.asarray(inputs["Wo"], dtype=np.float32)

    # Reference reshapes kv to [B,S,H,2*Dh]: per head, rows h*128..h*128+63 of
    # Wkv are the k-projection, rows h*128+64..h*128+127 the v-projection.
    k_rows = np.concatenate([np.arange(h * 128, h * 128 + Dh) for h in range(H)])
    v_rows = np.concatenate([np.arange(h * 128 + Dh, (h + 1) * 128)
                             for h in range(H)])
    WqT = np.ascontiguousarray(Wq.T)
    WkT = np.ascontiguousarray(Wkv[k_rows].T)
    WvT = np.ascontiguousarray(Wkv[v_rows].T)
    WoT = np.ascontiguousarray(Wo.T)

    in_maps = []
    for c in range(N_CORES):
        b, half = c // 2, c % 2
        xTc = np.ascontiguousarray(x[b, half * SQ:(half + 1) * SQ, :].T)
        yTb = np.ascontiguousarray(y[b].T)
        in_maps.append({"xT": xTc, "yT": yTb, "WqT": WqT, "WkT": WkT,
                        "WvT": WvT, "WoT": WoT})

    nc = _get_nc()
    res = run_bass_kernel_spmd(nc, in_maps, core_ids=list(range(N_CORES)),
                               trace=trace, trace_cores=trace_cores)
    out = np.empty((B, S, D), dtype=np.float32)
    for c in range(N_CORES):
        b, half = c // 2, c % 2
        out[b, half * SQ:(half + 1) * SQ, :] = res.results[c]["out"]
    return out, res


def kernel(**inputs) -> np.ndarray:
    mask = np.asarray(inputs["mask"], dtype=np.float32)
    bq = np.asarray(inputs["bq"], dtype=np.float32)
    bkv = np.asarray(inputs["bkv"], dtype=np.float32)
    bo = np.asarray(inputs["bo"], dtype=np.float32)
    if mask.any() or bq.any() or bkv.any() or bo.any():
        # Device kernel hardcodes zero mask/biases; stay correct regardless.
        return _host_fallback(
            np.asarray(inputs["x"], dtype=np.float32),
            np.asarray(inputs["y"], dtype=np.float32),
            mask, np.asarray(inputs["Wq"], dtype=np.float32), bq,
            np.asarray(inputs["Wkv"], dtype=np.float32), bkv,
            np.asarray(inputs["Wo"], dtype=np.float32), bo)
    out, _ = _run(inputs)
    return out



# revision 13
# speedup vs baseline: 1.2877x; 1.2877x over previous
"""Multi-head cross-attention Trainium2 kernel (8 NeuronCores, SPMD).

Problem: nn_MultiHeadCrossAttention_31791347925263
  x:[4,2048,768], y:[4,2048,768], 12 heads x 64, fp32.
  out = softmax((x Wq^T)(y Wk^T)^T / 8 + mask) (y Wv^T) Wo^T   (+ zero biases)

Sharding: 8 cores = (batch b in 0..3) x (query half in 0..1). Each core
computes the full attention for its 1024 query rows against all 2048 keys
of its batch. No collectives; outputs concatenate.

Design (v3, ACT-limited pipeline, all-bf16):
  Measured HW laws driving this shape:
  - A K=128 matmul streams 512 moving cols in ~226ns (full 2.4GHz);
    K<=64 matmuls run exactly 2x slower. So QK (contraction = head_dim
    = 64) uses ZERO-PADDED stationaries: kTz[h] is [128, 2048] with the
    head's k in rows 0-63 and zeros in 64-127; the moving qT block has
    the sibling head's (finite) q in rows 64-127, killed by the zeros.
    This halves QK's PE time.
  - fp8 anywhere in the PV chain costs ~2-3% output error (softmax
    output rel err ~= per-element rel err of P~/v; it does NOT average
    down), so everything stays bf16 (~0.7% total).
  - The Scalar engine exp (25.2M scores -> 192 x [128,1024] ACTIVATEs
    at ~1.1us) is a ~214us floor. PE work (614k cols ~= 256us at full
    clock) is brought to ~82+82+15us attention-side by the K=128 trick,
    with the 77us of kT/qT/v' projections injected into per-pair PE
    slack so ACT never starves. PSUM: 2x QK score slots [128,1024] (4
    banks) + PV accumulator [65,1024] (2) + 2 projection slots (2).
  - Single-head pipeline with QK emitted 2 key-blocks ahead of PV so
    the in-order PE queue never blocks on exp; projection chunks are
    force-drained before their consumers (deadlock safety).
  - PV's 65th stationary column (ones) accumulates the softmax
    denominator free; normalize = DVE copy + reciprocal + gpsimd
    partition-broadcast + DVE mul into separate bf16 vnorm tiles.
  - Output projection (bf16) at the end over 3 rotating PSUM slots.
"""

import numpy as np

B, S, D = 4, 2048, 768
H, Dh = 12, 64
SQ = S // 2          # queries per core
N_CORES = 8
DB = D // 128        # 6 d_model blocks
SKB = S // 128       # 16 key blocks
SQB = SQ // 128      # 8 query blocks per core
VPW = H * (Dh + 1)   # 780: v' width (64 v cols + 1 ones col per head)

_cache = {}


def _build_nc():
    import concourse.mybir as mybir
    import concourse.tile as tile
    from concourse import bacc

    f32 = mybir.dt.float32
    bf16 = mybir.dt.bfloat16
    EXP = mybir.ActivationFunctionType.Exp

    nc = bacc.Bacc("TRN2", target_bir_lowering=False)
    x16 = nc.dram_tensor("x16", [128, DB, SQ], bf16, kind="ExternalInput")
    y16 = nc.dram_tensor("y16", [128, DB, S], bf16, kind="ExternalInput")
    wq16 = nc.dram_tensor("wq16", [128, DB, D], bf16, kind="ExternalInput")
    wk16 = nc.dram_tensor("wk16", [128, DB, D], bf16, kind="ExternalInput")
    wv16 = nc.dram_tensor("wv16", [128, DB, D], bf16, kind="ExternalInput")
    wo16 = nc.dram_tensor("wo16", [128, DB, D], bf16, kind="ExternalInput")
    out = nc.dram_tensor("out", [SQ, D], f32, kind="ExternalOutput")

    with tile.TileContext(nc) as tc:
        with tc.tile_pool(name="persist", bufs=1) as pp, \
             tc.tile_pool(name="mmps", bufs=2, space="PSUM") as mm_ps, \
             tc.tile_pool(name="vtps", bufs=1, space="PSUM") as vt_ps, \
             tc.tile_pool(name="pjps", bufs=2, space="PSUM") as pj_ps, \
             tc.tile_pool(name="pt16p", bufs=4) as pt_pool, \
             tc.tile_pool(name="nrm", bufs=1) as nrm_pool, \
             tc.tile_pool(name="osb", bufs=3) as o_pool:

            y16t = pp.tile([128, DB, S], bf16, name="y16t")
            wk16t = pp.tile([128, DB, D], bf16, name="wk16t")
            x16t = pp.tile([128, DB, SQ], bf16, name="x16t")
            wq16t = pp.tile([128, DB, D], bf16, name="wq16t")
            wv16t = pp.tile([128, DB, D], bf16, name="wv16t")
            wo16t = pp.tile([128, DB, D], bf16, name="wo16t")

            # zero-padded per-head k: rows 0-63 = head's kT, 64-127 = 0
            kTz = [pp.tile([128, S], bf16, name=f"kTz{i}") for i in range(H)]
            qT = [pp.tile([128, SQ], bf16, name=f"qT{i}") for i in range(DB)]
            vnorm = [pp.tile([128, SQ], bf16, name=f"vn{i}")
                     for i in range(DB)]
            vp16 = [pp.tile([128, VPW], bf16, name=f"vp16_{i}")
                    for i in range(SKB)]
            vp3 = [t.rearrange("p (h c) -> p h c", c=Dh + 1) for t in vp16]

            # ---- input DMA, priority order ----
            nc.sync.dma_start(out=wk16t, in_=wk16[:, :, :])
            for kb in range(DB):
                nc.sync.dma_start(out=y16t[:, kb, :], in_=y16[:, kb, :])
            nc.sync.dma_start(out=wq16t, in_=wq16[:, :, :])
            for kb in range(DB):
                nc.sync.dma_start(out=x16t[:, kb, :], in_=x16[:, kb, :])
            nc.sync.dma_start(out=wv16t, in_=wv16[:, :, :])
            nc.sync.dma_start(out=wo16t, in_=wo16[:, :, :])

            # head h's k occupies the same partition rows as its q in qT:
            # even heads rows 0-63 (zeros 64-127), odd heads rows 64-127
            for h in range(H):
                z0 = 64 if h % 2 == 0 else 0
                nc.gpsimd.memset(kTz[h][z0:z0 + 64, :], 0.0)
            for skb in range(SKB):
                nc.vector.memset(vp3[skb][:, :, Dh], 1.0)

            # ---- projection chunk emitters ----
            def emit_kt_chunk(ob, c4):
                ps = pj_ps.tile([128, 512], f32, name="pjps", tag="pjps")
                for kb in range(DB):
                    nc.tensor.matmul(
                        ps[:, :],
                        wk16t[:, kb, ob * 128:(ob + 1) * 128],
                        y16t[:, kb, c4 * 512:(c4 + 1) * 512],
                        start=(kb == 0), stop=(kb == DB - 1))
                cols = slice(c4 * 512, (c4 + 1) * 512)
                nc.vector.tensor_copy(kTz[2 * ob][0:64, cols], ps[0:64, :])
                nc.vector.tensor_copy(kTz[2 * ob + 1][64:128, cols],
                                      ps[64:128, :])

            def emit_qt_chunk(ob, c2):
                ps = pj_ps.tile([128, 512], f32, name="pjps", tag="pjps")
                for kb in range(DB):
                    nc.tensor.matmul(
                        ps[:, :],
                        wq16t[:, kb, ob * 128:(ob + 1) * 128],
                        x16t[:, kb, c2 * 512:(c2 + 1) * 512],
                        start=(kb == 0), stop=(kb == DB - 1))
                nc.vector.tensor_copy(
                    qT[ob][:, c2 * 512:(c2 + 1) * 512], ps[:, :])

            def emit_vp_chunk(skb, c):
                ps = pj_ps.tile([128, 512], f32, name="pjps", tag="pjps")
                for kb in range(DB):
                    nc.tensor.matmul(
                        ps[:, 0:384],
                        y16t[:, kb, skb * 128:(skb + 1) * 128],
                        wv16t[:, kb, c * 384:(c + 1) * 384],
                        start=(kb == 0), stop=(kb == DB - 1))
                src = ps[:, 0:384].rearrange("p (h c) -> p h c", c=Dh)
                nc.vector.tensor_copy(
                    vp3[skb][:, c * 6:(c + 1) * 6, 0:Dh], src)

            # task queue: (tag, mm_count, emit_fn), in need-by order
            tasks = []
            for skb in range(SKB):
                tasks.append((("vp", skb, 0), 6,
                              lambda skb=skb: emit_vp_chunk(skb, 0)))
            for ob in (1, 2, 3):
                for c4 in range(4):
                    tasks.append((("kt", ob), 6,
                                  lambda ob=ob, c4=c4: emit_kt_chunk(ob, c4)))
                for c2 in range(2):
                    tasks.append((("qt", ob), 6,
                                  lambda ob=ob, c2=c2: emit_qt_chunk(ob, c2)))
            for skb in range(SKB):
                tasks.append((("vp", skb, 1), 6,
                              lambda skb=skb: emit_vp_chunk(skb, 1)))
            for ob in (4, 5):
                for c4 in range(4):
                    tasks.append((("kt", ob), 6,
                                  lambda ob=ob, c4=c4: emit_kt_chunk(ob, c4)))
                for c2 in range(2):
                    tasks.append((("qt", ob), 6,
                                  lambda ob=ob, c2=c2: emit_qt_chunk(ob, c2)))

            state = {"budget": 0.0}

            def force(pred):
                rest = []
                for t in tasks:
                    if pred(t[0]):
                        t[2]()
                        state["budget"] -= t[1]
                    else:
                        rest.append(t)
                tasks[:] = rest

            def inject(budget_add):
                state["budget"] += budget_add
                while tasks and tasks[0][1] <= state["budget"]:
                    tag, mms, fn = tasks.pop(0)
                    fn()
                    state["budget"] -= mms

            # ---- prelude: kTz[0,1], qT[0] ----
            for c4 in range(4):
                emit_kt_chunk(0, c4)
            for c2 in range(2):
                emit_qt_chunk(0, c2)

            # ---- attention pipeline (flat skb stream, PV lags QK by 2) ----
            pt_live = {}

            def emit_qk(h, s):
                hb = h // 2
                st = mm_ps.tile([128, SQ], f32, name="mmps", tag="mmps",
                                padded_shape=[128, SQ])
                for j in range(2):
                    nc.tensor.matmul(
                        st[:, j * 512:(j + 1) * 512],
                        kTz[h][:, s * 128:(s + 1) * 128],
                        qT[hb][:, j * 512:(j + 1) * 512],
                        start=True, stop=True)
                pt = pt_pool.tile([128, SQ], bf16, name="pt16")
                nc.scalar.activation(pt[:, :], st[:, :], EXP, scale=0.125)
                pt_live[(h, s)] = pt

            vt_live = {}

            def emit_pv(h, s):
                if s == 0:
                    vt_live[h] = vt_ps.tile([65, SQ], f32, name="valT",
                                            tag="valT",
                                            padded_shape=[128, SQ])
                vt = vt_live[h]
                pt = pt_live.pop((h, s))
                force(lambda t: t[0] == "vp" and t[1] == s
                      and t[2] == (0 if h < 6 else 1))
                for j in range(2):
                    nc.tensor.matmul(
                        vt[:, j * 512:(j + 1) * 512],
                        vp16[s][:, h * 65:h * 65 + 65],
                        pt[:, j * 512:(j + 1) * 512],
                        start=(s == 0), stop=(s == SKB - 1))

            def emit_vnorm(h):
                hb, r0 = h // 2, (h % 2) * 64
                vt = vt_live.pop(h)
                vals = nrm_pool.tile([65, SQ], f32, name="vals")
                nc.vector.tensor_copy(vals[:, :], vt[:, :])
                rec = nrm_pool.tile([1, SQ], f32, name="rec")
                nc.vector.reciprocal(rec[:, :], vals[64:65, :])
                rbc = nrm_pool.tile([64, SQ], f32, name="rbc")
                nc.gpsimd.partition_broadcast(rbc[:, :], rec[:, :])
                nc.vector.tensor_mul(
                    vnorm[hb][r0:r0 + 64, :], vals[0:64, :], rbc[:, :])

            NS = H * SKB  # 192 (h, s) units
            LAG = 2
            for u in range(NS + LAG):
                if u < NS:
                    h2, s2 = divmod(u, SKB)
                    if s2 == 0:
                        force(lambda t, hb2=h2 // 2:
                              t[0] in ("kt", "qt") and t[1] == hb2)
                    emit_qk(h2, s2)
                if u >= LAG:
                    h1, s1 = divmod(u - LAG, SKB)
                    emit_pv(h1, s1)
                    if s1 == SKB - 1:
                        emit_vnorm(h1)
                    inject(1.9)

            force(lambda t: True)

            # ---- output projection ----
            for sqb in range(SQB):
                if sqb % 3 < 2:
                    op = mm_ps.tile([128, D], f32, name="mmps", tag="mmps",
                                    padded_shape=[128, SQ])
                else:
                    op = vt_ps.tile([128, D], f32, name="valT", tag="valT",
                                    padded_shape=[128, SQ])
                for kb in range(DB):
                    for nc2 in range(2):
                        n0, n1 = nc2 * 512, min(D, (nc2 + 1) * 512)
                        nc.tensor.matmul(
                            op[:, n0:n1],
                            vnorm[kb][:, sqb * 128:(sqb + 1) * 128],
                            wo16t[:, kb, n0:n1],
                            start=(kb == 0), stop=(kb == DB - 1))
                ot = o_pool.tile([128, D], f32, name="osb")
                nc.vector.tensor_copy(ot[:, :], op[:, :])
                nc.sync.dma_start(
                    out=out[sqb * 128:(sqb + 1) * 128, :], in_=ot[:, :])

    nc.compile()
    return nc


def _get_nc():
    if "nc" not in _cache:
        _cache["nc"] = _build_nc()
    return _cache["nc"]


def _host_fallback(x, y, mask, Wq, bq, Wkv, bkv, Wo, bo):
    Bb, Ss, _ = x.shape
    q = x @ Wq.T + bq
    kv = y @ Wkv.T + bkv
    q = q.reshape(Bb, Ss, H, Dh).transpose(0, 2, 1, 3)
    kv = kv.reshape(Bb, Ss, H, 2 * Dh).transpose(0, 2, 1, 3)
    k, v = kv[..., :Dh], kv[..., Dh:]
    scaled = np.einsum("bhqd,bhkd->bhqk", q, k) / np.sqrt(np.float32(Dh))
    scaled = scaled + mask
    scaled -= scaled.max(axis=-1, keepdims=True)
    e = np.exp(scaled)
    attn = e / e.sum(axis=-1, keepdims=True)
    values = np.einsum("bhqk,bhkd->bhqd", attn, v)
    values = values.transpose(0, 2, 1, 3).reshape(Bb, Ss, H * Dh)
    return (values @ Wo.T + bo).astype(np.float32)


def _blk(mat_t, dtype):
    """[768, N] row-blocked to [128, 6, N] in the given ml dtype."""
    n = mat_t.shape[1]
    return np.ascontiguousarray(
        mat_t.reshape(DB, 128, n).transpose(1, 0, 2)).astype(dtype)


def _run(inputs, trace=False, trace_cores=None):
    """Returns (full_output, BassKernelResults)."""
    import ml_dtypes
    from concourse.bass_utils import run_bass_kernel_spmd

    bf16 = ml_dtypes.bfloat16

    x = np.ascontiguousarray(np.asarray(inputs["x"], dtype=np.float32))
    y = np.ascontiguousarray(np.asarray(inputs["y"], dtype=np.float32))
    Wq = np.asarray(inputs["Wq"], dtype=np.float32)
    Wkv = np.asarray(inputs["Wkv"], dtype=np.float32)
    Wo = np.asarray(inputs["Wo"], dtype=np.float32)

    # Reference reshapes kv to [B,S,H,2*Dh]: per head, rows h*128..h*128+63 of
    # Wkv are the k-projection, rows h*128+64..h*128+127 the v-projection.
    k_rows = np.concatenate([np.arange(h * 128, h * 128 + Dh) for h in range(H)])
    v_rows = np.concatenate([np.arange(h * 128 + Dh, (h + 1) * 128)
                             for h in range(H)])
    wq16 = _blk(Wq.T, bf16)
    wk16 = _blk(Wkv[k_rows].T, bf16)
    wv16 = _blk(Wkv[v_rows].T, bf16)
    wo16 = _blk(Wo.T, bf16)

    in_maps = []
    for c in range(N_CORES):
        b, half = c // 2, c % 2
        xT = x[b, half * SQ:(half + 1) * SQ, :].T
        yT = y[b].T
        in_maps.append({
            "x16": _blk(xT, bf16),
            "y16": _blk(yT, bf16),
            "wq16": wq16, "wk16": wk16, "wv16": wv16, "wo16": wo16,
        })

    nc = _get_nc()
    res = run_bass_kernel_spmd(nc, in_maps, core_ids=list(range(N_CORES)),
                               trace=trace, trace_cores=trace_cores)
    out = np.empty((B, S, D), dtype=np.float32)
    for c in range(N_CORES):
        b, half = c // 2, c % 2
        out[b, half * SQ:(half + 1) * SQ, :] = res.results[c]["out"]
    return out, res


def kernel(**inputs) -> np.ndarray:
    mask = np.asarray(inputs["mask"], dtype=np.float32)
    bq = np.asarray(inputs["bq"], dtype=np.float32)
    bkv = np.asarray(inputs["bkv"], dtype=np.float32)
    bo = np.asarray(inputs["bo"], dtype=np.float32)
    if mask.any() or bq.any() or bkv.any() or bo.any():
        # Device kernel hardcodes zero mask/biases; stay correct regardless.
        return _host_fallback(
            np.asarray(inputs["x"], dtype=np.float32),
            np.asarray(inputs["y"], dtype=np.float32),
            mask, np.asarray(inputs["Wq"], dtype=np.float32), bq,
            np.asarray(inputs["Wkv"], dtype=np.float32), bkv,
            np.asarray(inputs["Wo"], dtype=np.float32), bo)
    out, _ = _run(inputs)
    return out


# revision 21
# speedup vs baseline: 1.3368x; 1.0382x over previous
"""Multi-head cross-attention Trainium2 kernel (8 NeuronCores, SPMD).

Problem: nn_MultiHeadCrossAttention_31791347925263
  x:[4,2048,768], y:[4,2048,768], 12 heads x 64, fp32.
  out = softmax((x Wq^T)(y Wk^T)^T / 8 + mask) (y Wv^T) Wo^T   (+ zero biases)

Sharding: 8 cores = (batch b in 0..3) x (query half in 0..1). Each core
computes the full attention for its 1024 query rows against all 2048 keys
of its batch. No collectives; outputs concatenate.

Design (v3, ACT-limited pipeline, all-bf16):
  Measured HW laws driving this shape:
  - A K=128 matmul streams 512 moving cols in ~226ns (full 2.4GHz);
    K<=64 matmuls run exactly 2x slower. So QK (contraction = head_dim
    = 64) uses ZERO-PADDED stationaries: kTz[h] is [128, 2048] with the
    head's k in rows 0-63 and zeros in 64-127; the moving qT block has
    the sibling head's (finite) q in rows 64-127, killed by the zeros.
    This halves QK's PE time.
  - fp8 anywhere in the PV chain costs ~2-3% output error (softmax
    output rel err ~= per-element rel err of P~/v; it does NOT average
    down), so everything stays bf16 (~0.7% total).
  - The Scalar engine exp (25.2M scores -> 192 x [128,1024] ACTIVATEs
    at ~1.1us) is a ~214us floor. PE work (614k cols ~= 256us at full
    clock) is brought to ~82+82+15us attention-side by the K=128 trick,
    with the 77us of kT/qT/v' projections injected into per-pair PE
    slack so ACT never starves. PSUM: 2x QK score slots [128,1024] (4
    banks) + PV accumulator [65,1024] (2) + 2 projection slots (2).
  - Single-head pipeline with QK emitted 2 key-blocks ahead of PV so
    the in-order PE queue never blocks on exp; projection chunks are
    force-drained before their consumers (deadlock safety).
  - PV's 65th stationary column (ones) accumulates the softmax
    denominator free; normalize = DVE copy + reciprocal + gpsimd
    partition-broadcast + DVE mul into separate bf16 vnorm tiles.
  - Output projection (bf16) at the end over 3 rotating PSUM slots.
"""

import numpy as np

B, S, D = 4, 2048, 768
H, Dh = 12, 64
SQ = S // 2          # queries per core
N_CORES = 8
DB = D // 128        # 6 d_model blocks
SKB = S // 128       # 16 key blocks
SQB = SQ // 128      # 8 query blocks per core
VPW = H * (Dh + 1)   # 780: v' width (64 v cols + 1 ones col per head)

_cache = {}


def _build_nc():
    import concourse.mybir as mybir
    import concourse.tile as tile
    from concourse import bacc

    f32 = mybir.dt.float32
    bf16 = mybir.dt.bfloat16
    EXP = mybir.ActivationFunctionType.Exp

    nc = bacc.Bacc("TRN2", target_bir_lowering=False)
    x16 = nc.dram_tensor("x16", [128, DB, SQ], bf16, kind="ExternalInput")
    y16 = nc.dram_tensor("y16", [128, DB, S], bf16, kind="ExternalInput")
    wq16 = nc.dram_tensor("wq16", [128, DB, D], bf16, kind="ExternalInput")
    wk16 = nc.dram_tensor("wk16", [128, DB, D], bf16, kind="ExternalInput")
    wv16 = nc.dram_tensor("wv16", [128, DB, D], bf16, kind="ExternalInput")
    wo16 = nc.dram_tensor("wo16", [128, DB, D], bf16, kind="ExternalInput")
    out = nc.dram_tensor("out", [SQ, D], f32, kind="ExternalOutput")

    with tile.TileContext(nc) as tc:
        with tc.tile_pool(name="persist", bufs=1) as pp, \
             tc.tile_pool(name="mmps", bufs=2, space="PSUM") as mm_ps, \
             tc.tile_pool(name="vtps", bufs=1, space="PSUM") as vt_ps, \
             tc.tile_pool(name="pjps", bufs=2, space="PSUM") as pj_ps, \
             tc.tile_pool(name="pt16p", bufs=4) as pt_pool, \
             tc.tile_pool(name="nrm", bufs=1) as nrm_pool, \
             tc.tile_pool(name="osb", bufs=3) as o_pool:

            y16t = pp.tile([128, DB, S], bf16, name="y16t")
            wk16t = pp.tile([128, DB, D], bf16, name="wk16t")
            x16t = pp.tile([128, DB, SQ], bf16, name="x16t")
            wq16t = pp.tile([128, DB, D], bf16, name="wq16t")
            wv16t = pp.tile([128, DB, D], bf16, name="wv16t")
            wo16t = pp.tile([128, DB, D], bf16, name="wo16t")

            # zero-padded per-head k: rows 0-63 = head's kT, 64-127 = 0
            kTz = [pp.tile([128, S], bf16, name=f"kTz{i}") for i in range(H)]
            qT = [pp.tile([128, SQ], bf16, name=f"qT{i}") for i in range(DB)]
            vnorm = [pp.tile([128, SQ], bf16, name=f"vn{i}")
                     for i in range(DB)]
            vp16 = [pp.tile([128, VPW], bf16, name=f"vp16_{i}")
                    for i in range(SKB)]
            vp3 = [t.rearrange("p (h c) -> p h c", c=Dh + 1) for t in vp16]

            # ---- input DMA, priority order ----
            nc.sync.dma_start(out=wk16t, in_=wk16[:, :, :])
            for kb in range(DB):
                nc.sync.dma_start(out=y16t[:, kb, :], in_=y16[:, kb, :])
            nc.sync.dma_start(out=wq16t, in_=wq16[:, :, :])
            for kb in range(DB):
                nc.sync.dma_start(out=x16t[:, kb, :], in_=x16[:, kb, :])
            nc.sync.dma_start(out=wv16t, in_=wv16[:, :, :])
            nc.sync.dma_start(out=wo16t, in_=wo16[:, :, :])

            # head h's k occupies the same partition rows as its q in qT:
            # even heads rows 0-63 (zeros 64-127), odd heads rows 64-127
            for h in range(H):
                z0 = 64 if h % 2 == 0 else 0
                nc.gpsimd.memset(kTz[h][z0:z0 + 64, :], 0.0)
            for skb in range(SKB):
                nc.vector.memset(vp3[skb][:, :, Dh], 1.0)

            # ---- projection chunk emitters ----
            def emit_kt_chunk(ob, c4):
                ps = pj_ps.tile([128, 512], f32, name="pjps", tag="pjps")
                for kb in range(DB):
                    nc.tensor.matmul(
                        ps[:, :],
                        wk16t[:, kb, ob * 128:(ob + 1) * 128],
                        y16t[:, kb, c4 * 512:(c4 + 1) * 512],
                        start=(kb == 0), stop=(kb == DB - 1))
                cols = slice(c4 * 512, (c4 + 1) * 512)
                nc.vector.tensor_copy(kTz[2 * ob][0:64, cols], ps[0:64, :])
                nc.vector.tensor_copy(kTz[2 * ob + 1][64:128, cols],
                                      ps[64:128, :])

            def emit_qt_chunk(ob, c2):
                ps = pj_ps.tile([128, 512], f32, name="pjps", tag="pjps")
                for kb in range(DB):
                    nc.tensor.matmul(
                        ps[:, :],
                        wq16t[:, kb, ob * 128:(ob + 1) * 128],
                        x16t[:, kb, c2 * 512:(c2 + 1) * 512],
                        start=(kb == 0), stop=(kb == DB - 1))
                nc.vector.tensor_copy(
                    qT[ob][:, c2 * 512:(c2 + 1) * 512], ps[:, :])

            def emit_vp_chunk(skb, c):
                ps = pj_ps.tile([128, 512], f32, name="pjps", tag="pjps")
                for kb in range(DB):
                    nc.tensor.matmul(
                        ps[:, 0:384],
                        y16t[:, kb, skb * 128:(skb + 1) * 128],
                        wv16t[:, kb, c * 384:(c + 1) * 384],
                        start=(kb == 0), stop=(kb == DB - 1))
                src = ps[:, 0:384].rearrange("p (h c) -> p h c", c=Dh)
                nc.vector.tensor_copy(
                    vp3[skb][:, c * 6:(c + 1) * 6, 0:Dh], src)

            # task queue: (tag, mm_count, emit_fn), in need-by order
            tasks = []
            for skb in range(SKB):
                tasks.append((("vp", skb, 0), 6,
                              lambda skb=skb: emit_vp_chunk(skb, 0)))
            for ob in (1, 2, 3):
                for c4 in range(4):
                    tasks.append((("kt", ob), 6,
                                  lambda ob=ob, c4=c4: emit_kt_chunk(ob, c4)))
                for c2 in range(2):
                    tasks.append((("qt", ob), 6,
                                  lambda ob=ob, c2=c2: emit_qt_chunk(ob, c2)))
            for skb in range(SKB):
                tasks.append((("vp", skb, 1), 6,
                              lambda skb=skb: emit_vp_chunk(skb, 1)))
            for ob in (4, 5):
                for c4 in range(4):
                    tasks.append((("kt", ob), 6,
                                  lambda ob=ob, c4=c4: emit_kt_chunk(ob, c4)))
                for c2 in range(2):
                    tasks.append((("qt", ob), 6,
                                  lambda ob=ob, c2=c2: emit_qt_chunk(ob, c2)))

            state = {"budget": 0.0}

            def force(pred):
                rest = []
                for t in tasks:
                    if pred(t[0]):
                        t[2]()
                        state["budget"] -= t[1]
                    else:
                        rest.append(t)
                tasks[:] = rest

            def inject(budget_add):
                state["budget"] += budget_add
                while tasks and tasks[0][1] <= state["budget"]:
                    tag, mms, fn = tasks.pop(0)
                    fn()
                    state["budget"] -= mms

            # ---- prelude: kTz[0,1], qT[0] ----
            for c4 in range(4):
                emit_kt_chunk(0, c4)
            for c2 in range(2):
                emit_qt_chunk(0, c2)

            # ---- attention pipeline (flat skb stream, PV lags QK by 2) ----
            pt_live = {}

            def emit_qk(h, s):
                hb = h // 2
                st = mm_ps.tile([128, SQ], f32, name="mmps", tag="mmps",
                                padded_shape=[128, SQ])
                for j in range(2):
                    nc.tensor.matmul(
                        st[:, j * 512:(j + 1) * 512],
                        kTz[h][:, s * 128:(s + 1) * 128],
                        qT[hb][:, j * 512:(j + 1) * 512],
                        start=True, stop=True)
                pt = pt_pool.tile([128, SQ], bf16, name="pt16")
                nc.scalar.activation(pt[:, :], st[:, :], EXP, scale=0.125)
                pt_live[(h, s)] = pt

            vt_live = {}

            def emit_pv(h, s):
                if s == 0:
                    vt_live[h] = vt_ps.tile([65, SQ], f32, name="valT",
                                            tag="valT",
                                            padded_shape=[128, SQ])
                vt = vt_live[h]
                pt = pt_live.pop((h, s))
                force(lambda t: t[0] == "vp" and t[1] == s
                      and t[2] == (0 if h < 6 else 1))
                for j in range(2):
                    nc.tensor.matmul(
                        vt[:, j * 512:(j + 1) * 512],
                        vp16[s][:, h * 65:h * 65 + 65],
                        pt[:, j * 512:(j + 1) * 512],
                        start=(s == 0), stop=(s == SKB - 1))

            def emit_vnorm(h):
                hb, r0 = h // 2, (h % 2) * 64
                vt = vt_live.pop(h)
                vals = nrm_pool.tile([64, SQ], f32, name="vals")
                nc.vector.tensor_copy(vals[:, :], vt[0:64, :])
                den = nrm_pool.tile([1, SQ], f32, name="den")
                nc.vector.tensor_copy(den[:, :], vt[64:65, :])
                rec = nrm_pool.tile([1, SQ], f32, name="rec")
                # denominators are positive and well inside normal fp32 range;
                # the approx-fast custom op needs a partition-aligned source,
                # hence the den bounce off partition 64
                nc.vector.reciprocal_approx_fast(rec[:, :], den[:, :])
                rbc = nrm_pool.tile([64, SQ], f32, name="rbc")
                nc.gpsimd.partition_broadcast(rbc[:, :], rec[:, :])
                nc.vector.tensor_mul(
                    vnorm[hb][r0:r0 + 64, :], vals[:, :], rbc[:, :])

            NS = H * SKB  # 192 (h, s) units
            LAG = 2
            for u in range(NS + LAG):
                if u < NS:
                    h2, s2 = divmod(u, SKB)
                    if s2 == 0:
                        force(lambda t, hb2=h2 // 2:
                              t[0] in ("kt", "qt") and t[1] == hb2)
                    emit_qk(h2, s2)
                    inject(0.95)
                if u >= LAG:
                    h1, s1 = divmod(u - LAG, SKB)
                    emit_pv(h1, s1)
                    if s1 == SKB - 1:
                        emit_vnorm(h1)
                    inject(0.95)

            force(lambda t: True)

            # ---- output projection ----
            for sqb in range(SQB):
                if sqb % 3 < 2:
                    op = mm_ps.tile([128, D], f32, name="mmps", tag="mmps",
                                    padded_shape=[128, SQ])
                else:
                    op = vt_ps.tile([128, D], f32, name="valT", tag="valT",
                                    padded_shape=[128, SQ])
                for kb in range(DB):
                    for nc2 in range(2):
                        n0, n1 = nc2 * 512, min(D, (nc2 + 1) * 512)
                        nc.tensor.matmul(
                            op[:, n0:n1],
                            vnorm[kb][:, sqb * 128:(sqb + 1) * 128],
                            wo16t[:, kb, n0:n1],
                            start=(kb == 0), stop=(kb == DB - 1))
                ot = o_pool.tile([128, D], f32, name="osb")
                nc.vector.tensor_copy(ot[:, :], op[:, :])
                nc.sync.dma_start(
                    out=out[sqb * 128:(sqb + 1) * 128, :], in_=ot[:, :])

    nc.compile()
    return nc


def _get_nc():
    if "nc" not in _cache:
        _cache["nc"] = _build_nc()
    return _cache["nc"]


def _host_fallback(x, y, mask, Wq, bq, Wkv, bkv, Wo, bo):
    Bb, Ss, _ = x.shape
    q = x @ Wq.T + bq
    kv = y @ Wkv.T + bkv
    q = q.reshape(Bb, Ss, H, Dh).transpose(0, 2, 1, 3)
    kv = kv.reshape(Bb, Ss, H, 2 * Dh).transpose(0, 2, 1, 3)
    k, v = kv[..., :Dh], kv[..., Dh:]
    scaled = np.einsum("bhqd,bhkd->bhqk", q, k) / np.sqrt(np.float32(Dh))
    scaled = scaled + mask
    scaled -= scaled.max(axis=-1, keepdims=True)
    e = np.exp(scaled)
    attn = e / e.sum(axis=-1, keepdims=True)
    values = np.einsum("bhqk,bhkd->bhqd", attn, v)
    values = values.transpose(0, 2, 1, 3).reshape(Bb, Ss, H * Dh)
    return (values @ Wo.T + bo).astype(np.float32)


def _blk(mat_t, dtype):
    """[768, N] row-blocked to [128, 6, N] in the given ml dtype."""
    n = mat_t.shape[1]
    return np.ascontiguousarray(
        mat_t.reshape(DB, 128, n).transpose(1, 0, 2)).astype(dtype)


def _run(inputs, trace=False, trace_cores=None):
    """Returns (full_output, BassKernelResults)."""
    import ml_dtypes
    from concourse.bass_utils import run_bass_kernel_spmd

    bf16 = ml_dtypes.bfloat16

    x = np.ascontiguousarray(np.asarray(inputs["x"], dtype=np.float32))
    y = np.ascontiguousarray(np.asarray(inputs["y"], dtype=np.float32))
    Wq = np.asarray(inputs["Wq"], dtype=np.float32)
    Wkv = np.asarray(inputs["Wkv"], dtype=np.float32)
    Wo = np.asarray(inputs["Wo"], dtype=np.float32)

    # Reference reshapes kv to [B,S,H,2*Dh]: per head, rows h*128..h*128+63 of
    # Wkv are the k-projection, rows h*128+64..h*128+127 the v-projection.
    k_rows = np.concatenate([np.arange(h * 128, h * 128 + Dh) for h in range(H)])
    v_rows = np.concatenate([np.arange(h * 128 + Dh, (h + 1) * 128)
                             for h in range(H)])
    wq16 = _blk(Wq.T, bf16)
    wk16 = _blk(Wkv[k_rows].T, bf16)
    wv16 = _blk(Wkv[v_rows].T, bf16)
    wo16 = _blk(Wo.T, bf16)

    in_maps = []
    for c in range(N_CORES):
        b, half = c // 2, c % 2
        xT = x[b, half * SQ:(half + 1) * SQ, :].T
        yT = y[b].T
        in_maps.append({
            "x16": _blk(xT, bf16),
            "y16": _blk(yT, bf16),
            "wq16": wq16, "wk16": wk16, "wv16": wv16, "wo16": wo16,
        })

    nc = _get_nc()
    res = run_bass_kernel_spmd(nc, in_maps, core_ids=list(range(N_CORES)),
                               trace=trace, trace_cores=trace_cores)
    out = np.empty((B, S, D), dtype=np.float32)
    for c in range(N_CORES):
        b, half = c // 2, c % 2
        out[b, half * SQ:(half + 1) * SQ, :] = res.results[c]["out"]
    return out, res


def kernel(**inputs) -> np.ndarray:
    mask = np.asarray(inputs["mask"], dtype=np.float32)
    bq = np.asarray(inputs["bq"], dtype=np.float32)
    bkv = np.asarray(inputs["bkv"], dtype=np.float32)
    bo = np.asarray(inputs["bo"], dtype=np.float32)
    if mask.any() or bq.any() or bkv.any() or bo.any():
        # Device kernel hardcodes zero mask/biases; stay correct regardless.
        return _host_fallback(
            np.asarray(inputs["x"], dtype=np.float32),
            np.asarray(inputs["y"], dtype=np.float32),
            mask, np.asarray(inputs["Wq"], dtype=np.float32), bq,
            np.asarray(inputs["Wkv"], dtype=np.float32), bkv,
            np.asarray(inputs["Wo"], dtype=np.float32), bo)
    out, _ = _run(inputs)
    return out
